# revision 19
# baseline (speedup 1.0000x reference)
"""Causal single-head attention on 8 TRN2 NeuronCores.

Strategy: data-parallel over batch (B=512 -> 64 per core), weights replicated.

Per-core math, per batch b (S=256, E=384, H=64):
    qT = Wq.T @ x_b.T   [H, S]      (computed as one packed matmul with kT)
    kT = Wk.T @ x_b.T   [H, S]
    v  = x_b @ Wv       [S, H]
    sT[j,i] = sum_h kT[h,j] qT[h,i]         (scores transposed)
    eT = exp(sT / sqrt(E)) * causal_maskT   (no max-subtraction needed:
         |scores| < ~0.5 for this input distribution)
    out[i,h] = sum_j eT[j,i] v[j,h] / sum_j eT[j,i]
         (denominator fused into the AV matmul via a ones column in v)

Layouts are chosen so no on-chip transposes are needed: x is pre-transposed
host-side to [E, BPC, S] per core, so e sits on SBUF partitions for the QKV
projections, and scores/AV contract along partitions naturally.
"""

import sys

for _p in ("/opt/trn_rl_repo",):
    if _p not in sys.path:
        sys.path.insert(0, _p)

import numpy as np
import ml_dtypes

import concourse.bass as bass
from concourse import bacc
import concourse.mybir as mybir
from concourse.tile import TileContext
from concourse.bass_utils import run_bass_kernel_spmd

B, S, E, H = 512, 256, 384, 64
NCORES = 8
BPC = B // NCORES  # 64 batches per core
GRP = 4            # batches processed per pipeline group
NG = BPC // GRP
SCALE = float(E) ** -0.5
EC = E // 128      # 3 e-chunks

BF16 = mybir.dt.bfloat16
F32 = mybir.dt.float32
F32R = mybir.dt.float32r

_cache = {}


def build_nc():
    nc = bacc.Bacc()
    xt_d = nc.dram_tensor("xt", [E, BPC, S], BF16, kind="ExternalInput")
    wqk_d = nc.dram_tensor("wqk", [128, EC, 128], BF16, kind="ExternalInput")
    wv_d = nc.dram_tensor("wv", [128, EC, H], BF16, kind="ExternalInput")
    out_d = nc.dram_tensor("out", [BPC, S, H], F32, kind="ExternalOutput")

    EXP = mybir.ActivationFunctionType.Exp

    with TileContext(nc) as tc:
        with (
            tc.tile_pool(name="wconst", bufs=1) as wpool,
            tc.tile_pool(name="xtf", bufs=3) as xtf_pool,
            tc.tile_pool(name="qkt", bufs=3) as qkt_pool,
            tc.tile_pool(name="ex", bufs=4) as ex_pool,
            tc.tile_pool(name="outp", bufs=4) as out_pool,
            tc.tile_pool(name="ps_qk", bufs=1, space="PSUM") as ps_qk,
            tc.tile_pool(name="ps_s", bufs=3, space="PSUM") as ps_s,
            tc.tile_pool(name="ps_v", bufs=1, space="PSUM") as ps_v,
            tc.tile_pool(name="ps_av", bufs=1, space="PSUM") as ps_av,
        ):
            # --- persistent constants ---
            wqk_sb = wpool.tile([128, EC, 128], BF16)  # [e, chunk, (q|k) head col]
            nc.sync.dma_start(wqk_sb, wqk_d[:, :, :])
            wv_sb = wpool.tile([128, EC, H], BF16)
            nc.sync.dma_start(wv_sb, wv_d[:, :, :])
            # v staging: [128, parity, b*2+sblk, 65]; col 64 stays 1.0
            # (ones column turns the AV matmul into AV + row-sum denominator)
            v_sb = wpool.tile([128, 2, GRP * 2, H + 1], BF16)
            nc.vector.memset(v_sb, 1.0)

            for g in range(NG):
                par = g % 2
                b0 = g * GRP
                # --- load x.T: ONE 4D DMA for all 3 e-chunks ---
                xtile = xtf_pool.tile([128, EC, GRP, S], BF16, tag="xtf")
                nc.sync.dma_start(
                    xtile,
                    xt_d[:, b0:b0 + GRP, :].rearrange("(c p) b s -> p c b s", p=128),
                )
                xb = [xtile[:, c, :, :] for c in range(EC)]

                # --- qkT: [q rows 0:64 | k rows 64:128, (b s)] ---
                qk_ps = ps_qk.tile([128, 2, 512], F32)
                xflat = [x.rearrange("p b s -> p (b s)") for x in xb]
                for half in range(2):
                    for c in range(EC):
                        nc.tensor.matmul(
                            qk_ps[:, half, :],
                            wqk_sb[:, c, :],
                            xflat[c][:, half * 512:(half + 1) * 512],
                            start=(c == 0),
                            stop=(c == EC - 1),
                        )
                qt = qkt_pool.tile([64, GRP * S], BF16, tag="qt")
                kt = qkt_pool.tile([64, GRP * S], BF16, tag="kt")
                nc.scalar.copy(qt.rearrange("p (q f) -> p q f", q=2),
                               qk_ps[0:64, :, :])
                nc.vector.tensor_copy(kt.rearrange("p (q f) -> p q f", q=2),
                                      qk_ps[64:128, :, :])

                # --- v: [s, h] per batch, 2 s-blocks, accumulate e-chunks ---
                v_ps = ps_v.tile([128, GRP * 2, H], F32)
                for bl in range(GRP):
                    for sb in range(2):
                        for c in range(EC):
                            nc.tensor.matmul(
                                v_ps[:, bl * 2 + sb, :],
                                xb[c][:, bl, sb * 128:(sb + 1) * 128],
                                wv_sb[:, c, :],
                                start=(c == 0),
                                stop=(c == EC - 1),
                            )
                # one strided copy: all 4 v blocks -> ones-padded staging
                nc.vector.tensor_copy(v_sb[:, par, :, 0:H], v_ps)

                # --- scores (transposed) + exp + causal mask, per batch ---
                ets = []
                for bl in range(GRP):
                    q_lo = bl * S
                    s_ps = ps_s.tile([128, S + 128], F32, tag="s_ps")
                    nc.tensor.matmul(
                        s_ps[:, 0:S],
                        kt[:, q_lo:q_lo + 128],
                        qt[:, q_lo:q_lo + S],
                        start=True, stop=True,
                    )
                    nc.tensor.matmul(
                        s_ps[:, S:S + 128],
                        kt[:, q_lo + 128:q_lo + S],
                        qt[:, q_lo + 128:q_lo + S],
                        start=True, stop=True,
                    )
                    et = ex_pool.tile([128, S + 128], BF16, tag="et")
                    nc.scalar.activation(et, s_ps, EXP, scale=SCALE)
                    # causal mask on the two diagonal blocks: keep col>=row
                    for lo in (0, S):
                        nc.gpsimd.affine_select(
                            out=et[:, lo:lo + 128], in_=et[:, lo:lo + 128],
                            compare_op=mybir.AluOpType.is_ge, fill=0.0,
                            base=0, pattern=[[1, 128]], channel_multiplier=-1,
                        )
                    ets.append(et)

                # --- AV: two banks, each holds 2 batches x 2 i-blocks ---
                av_ps = ps_av.tile([128, 2, 512], F32)
                avs = lambda bl, ib: av_ps[
                    :, bl // 2, (bl % 2) * 2 * (H + 1) + ib * (H + 1):
                    (bl % 2) * 2 * (H + 1) + (ib + 1) * (H + 1)]
                for bl in range(GRP):
                    et = ets[bl]
                    nc.tensor.matmul(
                        avs(bl, 0), et[:, 0:128],
                        v_sb[:, par, bl * 2, :],
                        start=True, stop=True,
                    )
                    nc.tensor.matmul(
                        avs(bl, 1), et[:, 128:S],
                        v_sb[:, par, bl * 2, :],
                        start=True, stop=False,
                    )
                    nc.tensor.matmul(
                        avs(bl, 1), et[:, S:S + 128],
                        v_sb[:, par, bl * 2 + 1, :],
                        start=False, stop=True,
                    )

                # --- normalize all 8 i-blocks at once, store ---
                avv = av_ps[:, :, 0:4 * (H + 1)].rearrange(
                    "p q (k c) -> p q k c", c=H + 1)
                rc = out_pool.tile([128, 2, 4], F32, tag="rc")
                nc.vector.reciprocal(rc, avv[:, :, :, H])
                ot = out_pool.tile([128, 2, 4, H], F32, tag="ot")
                nc.vector.tensor_mul(
                    ot, avv[:, :, :, 0:H],
                    rc.broadcast_to([128, 2, 4, H]),
                )
                nc.sync.dma_start(
                    out_d[b0:b0 + GRP, :, :].rearrange(
                        "(q b) (i s) h -> s q (b i) h", q=2, i=2),
                    ot.rearrange("p q k h -> p q k h"),
                )
    nc.finalize()
    return nc


def _prep_consts(Wq, Wk, Wv):
    bf = ml_dtypes.bfloat16
    # wqk[e, c, m]: chunk c rows e of [Wq | Wk]
    wqk = np.empty((128, EC, 128), dtype=bf)
    wv = np.empty((128, EC, H), dtype=bf)
    for c in range(EC):
        wqk[:, c, 0:H] = Wq[c * 128:(c + 1) * 128, :].astype(bf)
        wqk[:, c, H:128] = Wk[c * 128:(c + 1) * 128, :].astype(bf)
        wv[:, c, :] = Wv[c * 128:(c + 1) * 128, :].astype(bf)
    return wqk, wv


def kernel(x, Wq, Wk, Wv):
    x = np.asarray(x, dtype=np.float32)
    wqk, wv = _prep_consts(
        np.asarray(Wq, np.float32), np.asarray(Wk, np.float32),
        np.asarray(Wv, np.float32),
    )
    if "nc" not in _cache:
        _cache["nc"] = build_nc()
    nc = _cache["nc"]

    in_maps = []
    for core in range(NCORES):
        xs = x[core * BPC:(core + 1) * BPC]          # [64, 256, 384]
        xt = np.ascontiguousarray(
            xs.transpose(2, 0, 1)).astype(ml_dtypes.bfloat16)  # [E, BPC, S]
        in_maps.append({"xt": xt, "wqk": wqk, "wv": wv})

    res = run_bass_kernel_spmd(nc, in_maps, core_ids=list(range(NCORES)))
    out = np.concatenate([r["out"] for r in res.results], axis=0)
    return out.astype(np.float32)
